# revision 13
# baseline (speedup 1.0000x reference)
"""DAF router kernel for 8 trn2 NeuronCores.

Self-contained: hardcodes shapes from the problem spec.
  h [16384, 4096] f32, metadata [16384, 2] f32, k=8,
  w1 [2,16], b1 [16], w2 [16,8], b2 [8], wg [4104, 64], bg [64], mu [64].
Returns (gating_weights [16384,64] f32, selected_indices [16384,8] i32, mu).

Sharding: token axis split across 8 cores (2048 tokens each); router params
replicated. Host pre-transposes h into a [128, C*T] chunk-major layout so
every device DMA is a fully contiguous per-partition slab with the
contraction dim (D) on SBUF partitions.

Modes (DAF_MODE):
  f32x2  - fp32 matmuls column-tiled 2x: even chunks -> PSUM partitions 0:64,
           odd chunks -> 64:128 (concurrent PE column groups); the halves are
           summed for free by accumulating two PE transposes into one PSUM
           tile. Exact (bit-level fp32 class).  [default]
  f32    - plain fp32 matmuls (4 cycles/row). Exact.
  bf16x3 - hi/lo bf16 split, 3 passes (~1e-5 logits err).
  f32r   - fp32r single pass (fast, reduced precision; ~92/16384 topk flips).
"""

import functools
import os

import numpy as np

N, D, E, KTOP = 16384, 4096, 64, 8
M_IN, M_H, M_OUT = 2, 16, 8
NCORES = 8
T = N // NCORES          # 2048 tokens per core
P = 128
C = D // P               # 32 contraction chunks
NG = T // 512            # 4 token groups of 512
JT = T // P              # 16 token tiles of 128
DMA_RAMP = (1, 1, 2, 4, 8, 8, 8)   # chunks per h DMA; sums to C
MAX_GRP = max(DMA_RAMP)

MODE = os.environ.get("DAF_MODE", "f32x2")
DEBUG_VARIANT = os.environ.get("DAF_DEBUG", "")  # "", "mmonly", "tronly", "tr2x"


def _build(mode: str, reps: int):
    import concourse.bass as bass  # noqa: F401
    import concourse.mybir as mybir
    import concourse.tile as tile
    from concourse import bacc
    from concourse.masks import make_identity

    dt = mybir.dt
    AF = mybir.ActivationFunctionType
    ALU = mybir.AluOpType

    h_dt = {"f32": dt.float32, "f32x2": dt.float32, "f32r": dt.float32r,
            "bf16x3": dt.bfloat16}[mode]
    split = mode == "bf16x3"
    coltile = mode == "f32x2"
    mm_coltile = coltile and DEBUG_VARIANT not in ("tronly", "tr2x")
    tr_double = (coltile and DEBUG_VARIANT != "mmonly") or DEBUG_VARIANT in ("tronly", "tr2x")

    nc = bacc.Bacc("TRN2", target_bir_lowering=False, debug=False, num_devices=NCORES)

    # --- DRAM I/O ---
    if split:
        h_hi = nc.dram_tensor("h_hi", [P, C * T], dt.bfloat16, kind="ExternalInput")
        h_lo = nc.dram_tensor("h_lo", [P, C * T], dt.bfloat16, kind="ExternalInput")
        wgh_hi = nc.dram_tensor("wgh_hi", [P, C * E], dt.bfloat16, kind="ExternalInput")
        wgh_lo = nc.dram_tensor("wgh_lo", [P, C * E], dt.bfloat16, kind="ExternalInput")
    else:
        h_in = nc.dram_tensor("h_in", [P, C * T], h_dt, kind="ExternalInput")
        wgh = nc.dram_tensor("wgh", [P, C * E], h_dt, kind="ExternalInput")
    mdT = nc.dram_tensor("mdT", [M_IN, T], dt.float32, kind="ExternalInput")
    w1 = nc.dram_tensor("w1", [M_IN, M_H], dt.float32, kind="ExternalInput")
    b1 = nc.dram_tensor("b1", [M_H, 1], dt.float32, kind="ExternalInput")
    w2 = nc.dram_tensor("w2", [M_H, M_OUT], dt.float32, kind="ExternalInput")
    b2 = nc.dram_tensor("b2", [M_OUT, 1], dt.float32, kind="ExternalInput")
    wg2_dt = dt.bfloat16 if split else (dt.float32r if mode == "f32r" else dt.float32)
    wg2 = nc.dram_tensor("wg2", [M_OUT, E], wg2_dt, kind="ExternalInput")
    bg = nc.dram_tensor("bg", [E, 1], dt.float32, kind="ExternalInput")
    gates = nc.dram_tensor("gates", [T, E], dt.float32, kind="ExternalOutput")
    sidx = nc.dram_tensor("sidx", [T, KTOP], dt.uint32, kind="ExternalOutput")

    with tile.TileContext(nc) as tc:
        with (
            tc.tile_pool(name="const", bufs=1) as const_pool,
            tc.tile_pool(name="hbuf", bufs=2) as h_pool,
            tc.tile_pool(name="work", bufs=2) as work_pool,
            tc.tile_pool(name="tok", bufs=3) as tok_pool,
            tc.tile_pool(name="outbuf", bufs=1) as out_pool,
            tc.tile_pool(name="lg_ps", bufs=1, space="PSUM") as lg_psum,
            tc.tile_pool(name="mlp_ps", bufs=2, space="PSUM") as mlp_psum,
            tc.tile_pool(name="tr_ps", bufs=2, space="PSUM") as tr_psum,
        ):
            # --- persistent constants / weights ---
            ident = const_pool.tile([P, P], dt.float32)
            make_identity(nc, ident[:])

            if split:
                wgh_hi_sb = const_pool.tile([P, C, E], dt.bfloat16)
                wgh_lo_sb = const_pool.tile([P, C, E], dt.bfloat16)
                nc.sync.dma_start(wgh_hi_sb[:], wgh_hi.rearrange("p (c e) -> p c e", c=C))
                nc.sync.dma_start(wgh_lo_sb[:], wgh_lo.rearrange("p (c e) -> p c e", c=C))
            else:
                wgh_sb = const_pool.tile([P, C, E], h_dt)
                nc.sync.dma_start(wgh_sb[:], wgh.rearrange("p (c e) -> p c e", c=C))
            mdT_sb = const_pool.tile([M_IN, T], dt.float32)
            nc.sync.dma_start(mdT_sb[:], mdT[:])
            w1_sb = const_pool.tile([M_IN, M_H], dt.float32)
            nc.sync.dma_start(w1_sb[:], w1[:])
            b1_sb = const_pool.tile([M_H, 1], dt.float32)
            nc.sync.dma_start(b1_sb[:], b1[:])
            w2_sb = const_pool.tile([M_H, M_OUT], dt.float32)
            nc.sync.dma_start(w2_sb[:], w2[:])
            b2_sb = const_pool.tile([M_OUT, 1], dt.float32)
            nc.sync.dma_start(b2_sb[:], b2[:])
            wg2_sb = const_pool.tile([M_OUT, E], wg2_dt)
            nc.sync.dma_start(wg2_sb[:], wg2[:])
            bg_sb = const_pool.tile([E, 1], dt.float32)
            nc.sync.dma_start(bg_sb[:], bg[:])

            def body():
                # metadata MLP per token group: m_embT [8, 512] per group
                m_embT_tiles = []
                for g in range(NG):
                    z1_ps = mlp_psum.tile([M_H, 512], dt.float32, tag="mlp")
                    nc.tensor.matmul(
                        z1_ps[:], w1_sb[:], mdT_sb[:, g * 512:(g + 1) * 512],
                        start=True, stop=True,
                    )
                    a1T = work_pool.tile([M_H, 512], dt.float32, tag="a1T")
                    nc.scalar.activation(a1T[:], z1_ps[:], AF.Gelu, bias=b1_sb[:])
                    me_ps = mlp_psum.tile([M_OUT, 512], dt.float32, tag="mlp")
                    nc.tensor.matmul(me_ps[:], w2_sb[:], a1T[:], start=True, stop=True)
                    if split:
                        m_embT = work_pool.tile([M_OUT, 512], dt.bfloat16, tag="membT")
                    elif mode == "f32r":
                        m_embT = work_pool.tile([M_OUT, 512], dt.float32r, tag="membT")
                    else:
                        m_embT = work_pool.tile([M_OUT, 512], dt.float32, tag="membT")
                    nc.scalar.activation(m_embT[:], me_ps[:], AF.Identity, bias=b2_sb[:])
                    m_embT_tiles.append(m_embT)

                # logitsT accumulation in PSUM
                lg_ps = lg_psum.tile([P, NG, 512], dt.float32)

                def do_chunk_mms(c, rhs_of):
                    first_lo = c == 0
                    first_hi = c == 1
                    for g in range(NG):
                        if mm_coltile:
                            half = (c % 2) * 64
                            out_ap = lg_ps[half:half + 64, g, :]
                            start = first_lo if half == 0 else first_hi
                            stop = (half == 64) and (c == C - 1)
                            nc.tensor.matmul(out_ap, wgh_sb[:, c, :], rhs_of(g),
                                             start=start, stop=stop)
                        elif split:
                            rh, rl = rhs_of(g)
                            nc.tensor.matmul(lg_ps[:E, g, :], wgh_hi_sb[:, c, :], rh,
                                             start=first_lo, stop=False)
                            nc.tensor.matmul(lg_ps[:E, g, :], wgh_hi_sb[:, c, :], rl,
                                             start=False, stop=False)
                            nc.tensor.matmul(lg_ps[:E, g, :], wgh_lo_sb[:, c, :], rh,
                                             start=False, stop=False)
                        else:
                            nc.tensor.matmul(lg_ps[:E, g, :], wgh_sb[:, c, :], rhs_of(g),
                                             start=first_lo, stop=False)

                c0 = 0
                for ngrp in DMA_RAMP:
                    if split:
                        hhi_t = h_pool.tile([P, MAX_GRP, T], dt.bfloat16, tag="hhi")
                        hlo_t = h_pool.tile([P, MAX_GRP, T], dt.bfloat16, tag="hlo")
                        nc.sync.dma_start(
                            hhi_t[:, :ngrp, :],
                            h_hi[:, c0 * T:(c0 + ngrp) * T].rearrange(
                                "p (c t) -> p c t", c=ngrp))
                        nc.sync.dma_start(
                            hlo_t[:, :ngrp, :],
                            h_lo[:, c0 * T:(c0 + ngrp) * T].rearrange(
                                "p (c t) -> p c t", c=ngrp))
                    else:
                        h_t = h_pool.tile([P, MAX_GRP, T], h_dt, tag="h")
                        nc.sync.dma_start(
                            h_t[:, :ngrp, :],
                            h_in[:, c0 * T:(c0 + ngrp) * T].rearrange(
                                "p (c t) -> p c t", c=ngrp))
                    for ci in range(ngrp):
                        c = c0 + ci
                        if split:
                            do_chunk_mms(c, lambda g, ci=ci, a=hhi_t, b=hlo_t: (
                                a[:, ci, g * 512:(g + 1) * 512],
                                b[:, ci, g * 512:(g + 1) * 512]))
                        else:
                            do_chunk_mms(c, lambda g, ci=ci, a=h_t:
                                         a[:, ci, g * 512:(g + 1) * 512])
                    c0 += ngrp

                # metadata contribution closes the (low-half) accumulation group
                for g in range(NG):
                    nc.tensor.matmul(lg_ps[:E, g, :], wg2_sb[:], m_embT_tiles[g][:],
                                     start=False, stop=not mm_coltile)

                # output accumulators
                gates_all = out_pool.tile([P, JT, E], dt.float32)
                sidx_all = out_pool.tile([P, JT, KTOP], dt.uint32)
                ltok_all = out_pool.tile([P, JT, E], dt.float32)
                vals8_all = out_pool.tile([P, JT, KTOP], dt.float32)

                for g in range(NG):
                    # PSUM -> SBUF, adding bg to the low half only
                    s_lo = work_pool.tile([P, 512], dt.float32, tag="slo")
                    nc.scalar.activation(s_lo[:E], lg_ps[:E, g, :], AF.Identity,
                                         bias=bg_sb[:])
                    if tr_double:
                        s_hi = work_pool.tile([P, 512], dt.float32, tag="shi")
                        if mm_coltile:
                            nc.scalar.copy(s_hi[64:128], lg_ps[64:128, g, :])
                        else:
                            nc.vector.memset(s_hi[:], 0.0)
                    for j4 in range(4):
                        j = g * 4 + j4
                        tr = tr_psum.tile([P, E], dt.float32, tag="tr")
                        # transpose(s) = s.T @ I; accumulating two transposes
                        # into one PSUM tile sums the chunk-parity halves.
                        nc.tensor.matmul(
                            tr[:], s_lo[:E, j4 * P:(j4 + 1) * P], ident[:E, :E],
                            is_transpose=True, start=True, stop=not tr_double)
                        if tr_double:
                            if DEBUG_VARIANT == "tr2x":
                                # accumulate a second transpose of zeros from
                                # base partition 0 (tests PSUM accum only)
                                nc.tensor.matmul(
                                    tr[:], s_hi[:E, j4 * P:(j4 + 1) * P],
                                    ident[:E, :E],
                                    is_transpose=True, start=False, stop=True)
                            else:
                                # transpose-mode from base partition 64 hangs
                                # the device; a regular matmul against the
                                # identity is an exact fp32 transpose too.
                                nc.tensor.matmul(
                                    tr[:], s_hi[64:128, j4 * P:(j4 + 1) * P],
                                    ident[64:128, 64:128],
                                    start=False, stop=True)
                        nc.scalar.copy(ltok_all[:, j], tr[:])
                        nc.vector.max(vals8_all[:, j], ltok_all[:, j])
                        nc.vector.max_index(sidx_all[:, j], vals8_all[:, j],
                                            ltok_all[:, j])

                # batched softmax epilogue (logits bounded, no max-shift needed)
                efull = out_pool.tile([P, JT, E], dt.float32)
                nc.scalar.activation(efull[:], ltok_all[:], AF.Exp)
                e8 = tok_pool.tile([P, JT, KTOP], dt.float32, tag="e8")
                nc.scalar.activation(e8[:], vals8_all[:], AF.Exp)
                ssum = tok_pool.tile([P, JT], dt.float32, tag="ssum")
                nc.vector.reduce_sum(ssum[:], e8[:], axis=mybir.AxisListType.X)
                rinv = tok_pool.tile([P, JT], dt.float32, tag="rinv")
                nc.vector.reciprocal(rinv[:], ssum[:])
                msk = out_pool.tile([P, JT, E], dt.float32)
                v8b = vals8_all[:, :, KTOP - 1:KTOP].to_broadcast([P, JT, E])
                nc.vector.tensor_tensor(msk[:], ltok_all[:], v8b, ALU.is_ge)
                nc.vector.tensor_tensor(msk[:], msk[:], efull[:], ALU.mult)
                rb = rinv[:, :, None].to_broadcast([P, JT, E])
                nc.vector.tensor_tensor(gates_all[:], msk[:], rb, ALU.mult)

                nc.sync.dma_start(
                    gates.rearrange("(j p) e -> p j e", p=P), gates_all[:])
                nc.sync.dma_start(
                    sidx.rearrange("(j p) e -> p j e", p=P), sidx_all[:])

            if reps == 1:
                body()
            else:
                with tc.For_i(0, reps, 1):
                    body()

    nc.compile()
    return nc


@functools.lru_cache(maxsize=4)
def _get_nc(mode: str, reps: int):
    return _build(mode, reps)


def _split_bf16(x: np.ndarray):
    import ml_dtypes

    hi = x.astype(ml_dtypes.bfloat16)
    lo = (x - hi.astype(np.float32)).astype(ml_dtypes.bfloat16)
    return hi, lo


def _chunk_major(hTc: np.ndarray) -> np.ndarray:
    """[D, T] -> [128, C*T] with chunk-major per-partition layout."""
    return np.ascontiguousarray(
        hTc.reshape(C, P, hTc.shape[1]).transpose(1, 0, 2)).reshape(P, -1)


def _prep_inputs(h, metadata, w1, b1, w2, b2, wg, bg, mode: str):
    h = np.asarray(h, np.float32)
    metadata = np.asarray(metadata, np.float32)
    wg = np.asarray(wg, np.float32)
    wgh_m = np.ascontiguousarray(
        wg[:D].reshape(C, P, E).transpose(1, 0, 2)).reshape(P, C * E)
    wg2_m = np.ascontiguousarray(wg[D:])
    common = {
        "w1": np.ascontiguousarray(np.asarray(w1, np.float32)),
        "b1": np.asarray(b1, np.float32).reshape(M_H, 1).copy(),
        "w2": np.ascontiguousarray(np.asarray(w2, np.float32)),
        "b2": np.asarray(b2, np.float32).reshape(M_OUT, 1).copy(),
        "bg": np.asarray(bg, np.float32).reshape(E, 1).copy(),
    }
    if mode == "bf16x3":
        common["wgh_hi"], common["wgh_lo"] = _split_bf16(wgh_m)
        common["wg2"] = _split_bf16(wg2_m)[0]  # bf16 is plenty for m_emb path
    else:
        common["wgh"] = wgh_m
        common["wg2"] = wg2_m

    hT = h.T  # [D, N] view
    mdT = metadata.T
    in_maps = []
    for core in range(NCORES):
        sl = slice(core * T, (core + 1) * T)
        m = dict(common)
        hcp = _chunk_major(np.ascontiguousarray(hT[:, sl]))
        if mode == "bf16x3":
            m["h_hi"], m["h_lo"] = _split_bf16(hcp)
        else:
            m["h_in"] = hcp
        m["mdT"] = np.ascontiguousarray(mdT[:, sl])
        in_maps.append(m)
    return in_maps


def run_on_device(h, metadata, w1, b1, w2, b2, wg, bg, mode: str = MODE,
                  reps: int = 1):
    """Run the bass kernel; returns (gates [N,E] f32, sidx [N,8] i32)."""
    from concourse.bass_utils import run_bass_kernel_spmd

    nc = _get_nc(mode, reps)
    in_maps = _prep_inputs(h, metadata, w1, b1, w2, b2, wg, bg, mode)
    res = run_bass_kernel_spmd(nc, in_maps, core_ids=list(range(NCORES)))
    gates = np.concatenate([res.results[i]["gates"] for i in range(NCORES)], axis=0)
    si = np.concatenate([res.results[i]["sidx"] for i in range(NCORES)], axis=0)
    return gates, si.astype(np.int32)


def kernel(h, metadata, k, w1, b1, w2, b2, wg, bg, mu):
    assert int(k) == KTOP
    gates, si = run_on_device(h, metadata, w1, b1, w2, b2, wg, bg, MODE, 1)
    return gates, si, np.asarray(mu, np.float32)
